# revision 1
# baseline (speedup 1.0000x reference)
"""Trainium2 Bass kernel for the CondConv-style dense CNN.

Model (per sample b):
  att[b]  = softmax(MLP(avgpool(scene_knowledge[b])) / 30)        # [16]
  agg_w   = sum_k att[b,k] * weight[k]                            # [256,256,3,3]
  out[b]  = conv3x3_same(x[b], agg_w) + att[b] @ bias + x[b]

Sharding: 8 cores = 4 sample-pairs (g) x 2 output-channel halves (h).
Each core processes 2 samples and 128 output channels.  All math runs
on-device; the host only does layout transforms + shard slicing.
"""

import sys
import numpy as np

sys.path.insert(0, "/opt/trn_rl_repo")

import concourse.bass as bass
import concourse.mybir as mybir
from concourse.tile import TileContext
from concourse.masks import make_identity

F32 = mybir.dt.float32
BF16 = mybir.dt.bfloat16
AX = mybir.AxisListType
OP = mybir.AluOpType
ACT = mybir.ActivationFunctionType

TEMPERATURE = 30.0
NCORES = 8


def build_program() -> bass.Bass:
    nc = bass.Bass()

    x2 = nc.declare_dram_parameter("x2", [2, 256, 64, 64], F32, isOutput=False)
    skv = nc.declare_dram_parameter("skv", [2, 3136], F32, isOutput=False)
    w1r = nc.declare_dram_parameter("w1r", [112, 28, 196], F32, isOutput=False)
    w2r = nc.declare_dram_parameter("w2r", [98, 2, 16], F32, isOutput=False)
    wt = nc.declare_dram_parameter("wt", [16, 2, 9, 128, 128], F32, isOutput=False)
    biash = nc.declare_dram_parameter("biash", [16, 128], F32, isOutput=False)
    selc = nc.declare_dram_parameter("selc", [2, 256], F32, isOutput=False)
    out2 = nc.declare_dram_parameter("out2", [2, 128, 64, 64], F32, isOutput=True)

    with TileContext(nc) as tc:
        with (
            tc.tile_pool(name="const", bufs=1) as cpool,
            tc.tile_pool(name="persist", bufs=1) as ppool,
            tc.tile_pool(name="wstream", bufs=6) as wpool,
            tc.tile_pool(name="xstage", bufs=2) as xpool,
            tc.tile_pool(name="outstage", bufs=8) as opool,
        ):
            # ---------------- small constant/param loads ----------------
            id_f32 = cpool.tile([16, 16], F32)
            make_identity(nc, id_f32)
            id_bf = cpool.tile([16, 16], BF16)
            make_identity(nc, id_bf)

            # sel[k, 128b:128b+128] = 1 if k == b else 0; used to broadcast
            # att row b across 128 partitions via PE.
            sel = cpool.tile([2, 256], F32)
            nc.sync.dma_start(out=sel, in_=selc[:])
            # DVE-copied twin so PE matmuls depend on a single engine (PE
            # instructions only support one sync wait).
            sel2 = cpool.tile([2, 256], F32)
            nc.vector.tensor_copy(sel2, sel)

            # Attention params stay fp32 and load via HWDGE (no SWDGE cast
            # dependency) so the attention chain finishes ASAP — it gates the
            # entire weight-mix stream.
            w1_sb = cpool.tile([112, 28, 196], F32)
            nc.sync.dma_start(out=w1_sb, in_=w1r[:])
            w2_sb = cpool.tile([98, 2, 16], F32)
            nc.sync.dma_start(out=w2_sb, in_=w2r[:])
            sk_sb = cpool.tile([112, 2, 28], F32)
            nc.sync.dma_start(
                out=sk_sb, in_=skv.rearrange("b (p c) -> p b c", p=112)
            )
            bias_sb = cpool.tile([16, 128], F32)
            nc.sync.dma_start(out=bias_sb, in_=biash[:])
            bias_sb2 = cpool.tile([16, 128], F32)
            nc.vector.tensor_copy(bias_sb2, bias_sb)

            att_bc = []  # [128, 16] f32 per sample: att[b, k] broadcast
            bias_b = ppool.tile([128, 2], F32)  # aggregated bias per sample

            with tc.tile_pool(name="psA", bufs=2, space="PSUM") as psA:
                # ---- hidden = relu(pooled @ w1.T) for both samples ----
                ps_h = psA.tile([2, 196], F32, tag="att_ps")
                for c in range(28):
                    nc.tensor.matmul(
                        ps_h,
                        sk_sb[:, :, c],          # [112, 2]
                        w1_sb[:, c, :],          # [112, 196]
                        start=(c == 0),
                        stop=(c == 27),
                    )
                hdn = ppool.tile([2, 196], F32)
                nc.vector.tensor_relu(hdn, ps_h)

                # transpose hdn chunks: [2, 98] -> [98, 2]
                hdnT = ppool.tile([98, 2, 2], F32)
                for c2 in range(2):
                    ps_t = psA.tile([98, 2], F32, tag="att_ps", name="ps_t")
                    nc.tensor.transpose(
                        ps_t, hdn[:, 98 * c2 : 98 * (c2 + 1)], id_f32[:2, :2]
                    )
                    nc.vector.tensor_copy(hdnT[:, c2, :], ps_t)

                # logits = hdn @ w2.T   -> [2, 16]
                ps_l = psA.tile([2, 16], F32, tag="att_ps", name="ps_l")
                for c2 in range(2):
                    nc.tensor.matmul(
                        ps_l,
                        hdnT[:, c2, :],          # [98, 2]
                        w2_sb[:, c2, :],         # [98, 16]
                        start=(c2 == 0),
                        stop=(c2 == 1),
                    )

                # softmax(logits / T) in f32 (ACT must stay off PSUM so conv
                # PSUM-bank reuse deps stay single-engine)
                logit_sb = ppool.tile([2, 16], F32)
                nc.vector.tensor_copy(logit_sb, ps_l)
                mx = ppool.tile([2, 1], F32)
                nc.vector.tensor_reduce(mx, logit_sb, axis=AX.X, op=OP.max)
                mxs = ppool.tile([2, 1], F32)
                nc.vector.tensor_scalar_mul(mxs, mx, -1.0 / TEMPERATURE)
                att_e = ppool.tile([2, 16], F32)
                nc.scalar.activation(
                    att_e, logit_sb, ACT.Exp, bias=mxs, scale=1.0 / TEMPERATURE
                )
                sm = ppool.tile([2, 1], F32)
                nc.vector.tensor_reduce(sm, att_e, axis=AX.X, op=OP.add)
                rec = ppool.tile([2, 1], F32)
                nc.vector.reciprocal(rec, sm)
                att_sb = ppool.tile([2, 16], F32)
                nc.vector.tensor_scalar_mul(att_sb, att_e, rec)

                # broadcast att rows across partitions: [128, 16] per sample
                for b in range(2):
                    ps_bc = psA.tile([128, 16], F32, tag="att_ps", name="ps_bc")
                    nc.tensor.matmul(
                        ps_bc, sel2[:, 128 * b : 128 * (b + 1)], att_sb,
                        start=True, stop=True,
                    )
                    abc = ppool.tile([128, 16], F32, name=f"att_bc{b}")
                    nc.vector.tensor_copy(abc, ps_bc)
                    att_bc.append(abc)

                # aggregated bias: bias_b[:, b] = sum_k att[b,k] bias[k, :]
                ps_at = psA.tile([16, 2], F32, tag="att_ps", name="ps_at")
                nc.tensor.transpose(ps_at, att_sb, id_f32[:2, :2])
                attT = ppool.tile([16, 2], F32)
                nc.vector.tensor_copy(attT, ps_at)
                ps_ab = psA.tile([128, 2], F32, tag="att_ps", name="ps_ab")
                nc.tensor.matmul(ps_ab, bias_sb2, attT, start=True, stop=True)
                nc.vector.tensor_copy(bias_b, ps_ab)

                # Age the att_bc/bias_b writes in the DVE stream: the first
                # weight-mix op below may carry only ONE sync wait (the slab
                # DMA), so its att_bc dep must be ≥ the DVE queue depth (8)
                # instructions old by the time it issues.
                age = ppool.tile([1, 16], F32, name="age")
                for j in range(8):
                    src = att_bc[j % 2] if j < 6 else bias_b
                    nc.vector.tensor_copy(age[:, j : j + 1], src[0:1, 0:1])

            # ---------------- x loads + zero-padded bf16 copies ----------------
            # xres[b]: f32 [128, 4096] for the residual (ci=0 == this core's
            # output-channel half, host-permuted).  xpad[b][ci]: bf16 [128,66,66].
            xres = []
            xpad = [[None, None], [None, None]]

            def load_x(b, ci):
                if ci == 0:
                    src_tile = ppool.tile([128, 4096], F32, name=f"xres{b}")
                    xres.append(src_tile)
                else:
                    src_tile = xpool.tile([128, 4096], F32, tag="xstg", name="xstg")
                nc.sync.dma_start(out=src_tile, in_=x2[b, 128 * ci : 128 * (ci + 1)])
                pad = ppool.tile([128, 66, 66], BF16, name=f"xpad{b}{ci}")
                nc.vector.memset(pad[:, 0, :], 0.0)
                nc.vector.memset(pad[:, 65, :], 0.0)
                nc.vector.memset(pad[:, 1:65, 0], 0.0)
                nc.vector.memset(pad[:, 1:65, 65], 0.0)
                # gpsimd (idle during the stream phase) does the cast-copy so
                # DVE stays dedicated to pacing the weight mix.
                nc.gpsimd.tensor_copy(
                    pad[:, 1:65, 1:65],
                    src_tile.rearrange("p (r c) -> p r c", r=64),
                )
                xpad[b][ci] = pad

            # ---------------- streamed weight aggregation ----------------
            # acc[b][ci][il, t, o] = sum_k att[b,k] * Wt[k, ci, t, il, o]
            acc = [[None, None], [None, None]]
            for b in range(2):
                for ci in range(2):
                    acc[b][ci] = ppool.tile([128, 9, 128], BF16, name=f"acc{b}{ci}")


            def stream_ci(ci):
                for b in range(2):
                    load_x(b, ci)
                for k in range(16):
                    wslab = wpool.tile([128, 9, 128], BF16, tag="wslab", name="wslab")
                    nc.gpsimd.dma_start(
                        out=wslab, in_=wt[k, ci].rearrange("t il o -> il t o")
                    )
                    # Weighted sum on DVE: TS-mul runs in 4x mode (360ns),
                    # TT-add in 2x mode (660ns).
                    for b in range(2):
                        if k == 0:
                            nc.vector.tensor_scalar_mul(
                                acc[b][ci], wslab, att_bc[b][:, 0:1]
                            )
                        else:
                            tmp = wpool.tile(
                                [128, 9, 128], BF16, tag="wtmp", name="wtmp", bufs=4
                            )
                            nc.vector.tensor_scalar_mul(
                                tmp, wslab, att_bc[b][:, k : k + 1]
                            )
                            nc.vector.tensor_add(acc[b][ci], acc[b][ci], tmp)

            stream_ci(0)
            stream_ci(1)

            # ---------------- conv + epilogue ----------------
            with tc.tile_pool(name="psC", bufs=8, space="PSUM") as psC:
                for blk in range(2):          # pixel-tile blocks: rows 0-31, 32-63
                    pts = range(4 * blk, 4 * blk + 4)
                    pcv = {}
                    # phase A: ci=0 taps (available early while ci=1 streams)
                    for b in range(2):
                        for pt in pts:
                            r0 = 8 * pt
                            p = psC.tile([128, 512], F32, tag="cv", name="pcv")
                            pcv[(b, pt)] = p
                            for t in range(9):
                                ty, tx = t // 3, t % 3
                                nc.tensor.matmul(
                                    p,
                                    acc[b][0][:, t, :],
                                    xpad[b][0][:, r0 + ty : r0 + ty + 8, tx : tx + 64],
                                    start=(t == 0),
                                    stop=False,
                                )
                    # phase B: ci=1 taps + epilogue
                    for b in range(2):
                        for pt in pts:
                            r0 = 8 * pt
                            p = pcv[(b, pt)]
                            for t in range(9):
                                ty, tx = t // 3, t % 3
                                nc.tensor.matmul(
                                    p,
                                    acc[b][1][:, t, :],
                                    xpad[b][1][:, r0 + ty : r0 + ty + 8, tx : tx + 64],
                                    start=False,
                                    stop=(t == 8),
                                )
                            osb = opool.tile([128, 512], F32, tag="osb", name="osb")
                            nc.scalar.activation(
                                osb, p, ACT.Identity, bias=bias_b[:, b : b + 1]
                            )
                            nc.gpsimd.tensor_tensor(
                                osb, osb, xres[b][:, 512 * pt : 512 * (pt + 1)], OP.add
                            )
                            nc.sync.dma_start(
                                out=out2[b, :, r0 : r0 + 8, :], in_=osb
                            )

    _split_multiwaits(nc)
    return nc


def _split_multiwaits(nc: bass.Bass):
    """This walrus build gives every TPB instruction exactly ONE sync-wait
    slot.  Tile emits multi-wait instructions; split the extras onto
    same-engine NoOp carriers inserted immediately before."""
    import bass_rust

    cnt = 0
    for fn in nc.m.functions:
        for blk in fn.blocks:
            out = []
            for ins in blk.instructions:
                si = getattr(ins, "sync_info", None)
                if si is not None and len(si.on_wait) > 1:
                    waits = list(si.on_wait)
                    for w in waits[:-1]:
                        cnt += 1
                        out.append(
                            bass_rust.InstNoOp(
                                name=f"waitcarrier-{cnt}",
                                engine=ins.engine,
                                ins=[],
                                outs=[],
                                sync_info=mybir.SyncInfo(
                                    on_wait=[w], on_update=[]
                                ),
                            )
                        )
                    ins.sync_info = mybir.SyncInfo(
                        on_wait=[waits[-1]], on_update=list(si.on_update)
                    )
                out.append(ins)
            blk.instructions = out


_PROGRAM = None


def _get_program():
    global _PROGRAM
    if _PROGRAM is None:
        _PROGRAM = build_program()
    return _PROGRAM


def _prepare_in_maps(x, scene_knowledge, weight, bias, att_w1, att_w2):
    x = np.ascontiguousarray(x, dtype=np.float32)
    scene_knowledge = np.ascontiguousarray(scene_knowledge, dtype=np.float32)
    weight = np.ascontiguousarray(weight, dtype=np.float32)
    bias = np.ascontiguousarray(bias, dtype=np.float32)
    att_w1 = np.ascontiguousarray(att_w1, dtype=np.float32)
    att_w2 = np.ascontiguousarray(att_w2, dtype=np.float32)

    K = 16
    # Wt[k, ci, t, il, o] = weight[k, o, 128*ci + il, ty*3+tx]
    Wt = np.ascontiguousarray(
        weight.reshape(K, 256, 2, 128, 9).transpose(0, 2, 4, 3, 1)
    )

    # fold 2x2 avg-pool into w1:  w1p[j, r*56+c] = 0.25 * w1[j, r//2, c//2]
    w1p = 0.25 * np.repeat(
        np.repeat(att_w1.reshape(196, 28, 28), 2, axis=1), 2, axis=2
    ).reshape(196, 3136)
    # w1r[p, c, j] = w1p[j, p*28 + c]
    w1r = np.ascontiguousarray(w1p.T.reshape(112, 28, 196))
    # w2r[p, c2, e] = att_w2[e, c2*98 + p]
    w2r = np.ascontiguousarray(att_w2.T.reshape(2, 98, 16).transpose(1, 0, 2))

    sel = np.zeros((2, 256), np.float32)
    sel[0, :128] = 1.0
    sel[1, 128:] = 1.0

    in_maps = []
    for c in range(NCORES):
        g, h = c // 2, c % 2
        perm = [h, 1 - h]  # i-chunk 0 == this core's output half (residual)
        x_core = np.ascontiguousarray(
            x[2 * g : 2 * g + 2].reshape(2, 2, 128, 64, 64)[:, perm]
        ).reshape(2, 256, 64, 64)
        wt_core = np.ascontiguousarray(
            Wt[:, perm][:, :, :, :, 128 * h : 128 * (h + 1)]
        )
        in_maps.append(
            {
                "x2": x_core,
                "skv": np.ascontiguousarray(
                    scene_knowledge[2 * g : 2 * g + 2].reshape(2, 3136)
                ),
                "w1r": w1r,
                "w2r": w2r,
                "wt": wt_core,
                "biash": np.ascontiguousarray(bias[:, 128 * h : 128 * (h + 1)]),
                "selc": sel,
            }
        )
    return in_maps


def _assemble(results):
    out = np.empty((8, 256, 64, 64), np.float32)
    for c in range(NCORES):
        g, h = c // 2, c % 2
        out[2 * g : 2 * g + 2, 128 * h : 128 * (h + 1)] = results[c]["out2"]
    return out


def run(inputs: dict, trace: bool = False, tmpdir: str | None = None):
    from concourse.bass_utils import run_bass_kernel_spmd

    nc = _get_program()
    in_maps = _prepare_in_maps(**inputs)
    res = run_bass_kernel_spmd(
        nc, in_maps, core_ids=list(range(NCORES)), trace=trace, tmpdir=tmpdir
    )
    return _assemble(res.results), res


def kernel(**inputs) -> np.ndarray:
    out, _ = run(inputs, trace=False)
    return out



# revision 3
# speedup vs baseline: 1.3400x; 1.3400x over previous
"""Trainium2 Bass kernel for the CondConv-style dense CNN.

Model (per sample b):
  att[b]  = softmax(MLP(avgpool(scene_knowledge[b])) / 30)        # [16]
  agg_w   = sum_k att[b,k] * weight[k]                            # [256,256,3,3]
  out[b]  = conv3x3_same(x[b], agg_w) + att[b] @ bias + x[b]

Sharding: 8 cores = 4 sample-pairs (g) x 2 output-channel halves (h).
Each core processes 2 samples and 128 output channels.

v2 design:
  - Host pre-casts W to bf16 in conv-ready layout and pre-pads x to the
    bf16 [66,66] padded layout -> all loads are plain HWDGE DMAs, no
    device-side casts, no SWDGE.
  - Attention: 2x2 avg-pool on DVE (2 tensor adds), 28 accumulating
    matmuls for the hidden layer, softmax, PE broadcast.
  - Weight mix: one fused scalar_tensor_tensor per expert slab
    (acc = w_k * att_k + acc) on DVE.
  - Conv: 2 passes (ci=0 / ci=1) x 2 samples x 8 pixel tiles, t-major
    for stationary reuse.  Pass-A PSUM is evacuated to SBUF staging so
    banks recycle; pass-B epilogue adds staging + bias + residual.
"""

import sys
import numpy as np

sys.path.insert(0, "/opt/trn_rl_repo")

import ml_dtypes
import concourse.bass as bass
import concourse.mybir as mybir
from concourse.tile import TileContext
from concourse.masks import make_identity

F32 = mybir.dt.float32
BF16 = mybir.dt.bfloat16
AX = mybir.AxisListType
OP = mybir.AluOpType
ACT = mybir.ActivationFunctionType

TEMPERATURE = 30.0
NCORES = 8
BF = ml_dtypes.bfloat16


def build_program() -> bass.Bass:
    nc = bass.Bass()

    # [b, ci, il, 66, 66] zero-padded bf16 x (ci=0 == this core's out half)
    xp = nc.declare_dram_parameter("xp", [2, 2, 128, 66, 66], BF16, isOutput=False)
    # scene knowledge packed for on-device 2x2 pooling:
    # skp[r, b, dr, cc, dc] = scene[b, 0, 2r+dr, 2cc+dc]
    skp = nc.declare_dram_parameter("skp", [28, 2, 2, 28, 2], F32, isOutput=False)
    # w1rr[r, cc, j] = 0.25 * att_w1[j, r*28 + cc]
    w1rr = nc.declare_dram_parameter("w1rr", [28, 28, 196], BF16, isOutput=False)
    # w2r[p, c2, e] = att_w2[e, c2*98 + p]
    w2r = nc.declare_dram_parameter("w2r", [98, 2, 16], F32, isOutput=False)
    # wtb[ci, kh, il, k2, t, o] = weight[kh*8+k2, oh, ci-chan il, t]
    wtb = nc.declare_dram_parameter("wtb", [2, 2, 128, 8, 9, 128], BF16, isOutput=False)
    biash = nc.declare_dram_parameter("biash", [16, 128], F32, isOutput=False)
    selc = nc.declare_dram_parameter("selc", [2, 256], F32, isOutput=False)
    out2 = nc.declare_dram_parameter("out2", [2, 128, 64, 64], F32, isOutput=True)

    with TileContext(nc) as tc:
        with (
            tc.tile_pool(name="const", bufs=1) as cpool,
            tc.tile_pool(name="persist", bufs=1) as ppool,
            tc.tile_pool(name="astage", bufs=16) as apool,
            tc.tile_pool(name="outstage", bufs=4) as opool,
        ):
            # ---------------- DMAs (two HWDGE rings) ----------------
            # sync ring: attention smalls first, then the 4 weight slabs.
            sk_sb = cpool.tile([28, 2, 2, 28, 2], F32)
            nc.sync.dma_start(out=sk_sb, in_=skp[:])
            w2_sb = cpool.tile([98, 2, 16], F32)
            nc.sync.dma_start(out=w2_sb, in_=w2r[:])
            sel = cpool.tile([2, 256], F32)
            nc.sync.dma_start(out=sel, in_=selc[:])
            bias_sb = cpool.tile([16, 128], F32)
            nc.sync.dma_start(out=bias_sb, in_=biash[:])

            wsb = []
            for ci in range(2):
                w = ppool.tile([128, 2, 8, 9, 128], BF16, name=f"wsb{ci}")
                for kh in range(2):
                    nc.sync.dma_start(out=w[:, kh], in_=wtb[ci, kh])
                wsb.append(w)

            # scalar ring: w1 then the 4 padded-x slabs.
            w1_sb = cpool.tile([28, 28, 196], BF16)
            nc.scalar.dma_start(out=w1_sb, in_=w1rr[:])
            xpad = [[None, None], [None, None]]
            for ci in range(2):
                for b in range(2):
                    t = ppool.tile([128, 66, 66], BF16, name=f"xp{b}{ci}")
                    nc.scalar.dma_start(out=t, in_=xp[b, ci])
                    xpad[b][ci] = t

            # DVE twins so PE matmuls get single-engine deps.
            sel2 = cpool.tile([2, 256], F32)
            nc.vector.tensor_copy(sel2, sel)
            bias_sb2 = cpool.tile([16, 128], F32)
            nc.vector.tensor_copy(bias_sb2, bias_sb)

            id_f32 = cpool.tile([16, 16], F32)
            make_identity(nc, id_f32)

            att_bc = []  # [128, 16] f32 per sample
            bias_b = ppool.tile([128, 2], F32)

            # ---------------- attention ----------------
            with tc.tile_pool(name="psA", bufs=2, space="PSUM") as psA:
                # 2x2 avg pool on DVE (0.25 folded into w1rr)
                pool_a = ppool.tile([28, 2, 28, 2], F32)
                nc.vector.tensor_add(pool_a, sk_sb[:, :, 0], sk_sb[:, :, 1])
                pooled = ppool.tile([28, 2, 28], BF16)
                nc.vector.tensor_add(pooled, pool_a[:, :, :, 0], pool_a[:, :, :, 1])

                # hidden = relu(pooled @ w1.T): 28 accumulating matmuls
                ps_h = psA.tile([2, 196], F32, tag="att_ps")
                for cc in range(28):
                    nc.tensor.matmul(
                        ps_h,
                        pooled[:, :, cc],        # [28, 2]
                        w1_sb[:, cc, :],         # [28, 196]
                        start=(cc == 0),
                        stop=(cc == 27),
                    )
                hdn = ppool.tile([2, 196], F32)
                nc.vector.tensor_relu(hdn, ps_h)

                # transpose hdn chunks: [2, 98] -> [98, 2]
                hdnT = ppool.tile([98, 2, 2], F32)
                for c2 in range(2):
                    ps_t = psA.tile([98, 2], F32, tag="att_ps", name="ps_t")
                    nc.tensor.transpose(
                        ps_t, hdn[:, 98 * c2 : 98 * (c2 + 1)], id_f32[:2, :2]
                    )
                    nc.vector.tensor_copy(hdnT[:, c2, :], ps_t)

                # logits = hdn @ w2.T -> [2, 16]
                ps_l = psA.tile([2, 16], F32, tag="att_ps", name="ps_l")
                for c2 in range(2):
                    nc.tensor.matmul(
                        ps_l,
                        hdnT[:, c2, :],
                        w2_sb[:, c2, :],
                        start=(c2 == 0),
                        stop=(c2 == 1),
                    )

                # softmax(logits / T) in f32
                logit_sb = ppool.tile([2, 16], F32)
                nc.vector.tensor_copy(logit_sb, ps_l)
                mx = ppool.tile([2, 1], F32)
                nc.vector.tensor_reduce(mx, logit_sb, axis=AX.X, op=OP.max)
                mxs = ppool.tile([2, 1], F32)
                nc.vector.tensor_scalar_mul(mxs, mx, -1.0 / TEMPERATURE)
                att_e = ppool.tile([2, 16], F32)
                nc.scalar.activation(
                    att_e, logit_sb, ACT.Exp, bias=mxs, scale=1.0 / TEMPERATURE
                )
                sm = ppool.tile([2, 1], F32)
                nc.vector.tensor_reduce(sm, att_e, axis=AX.X, op=OP.add)
                rec = ppool.tile([2, 1], F32)
                nc.vector.reciprocal(rec, sm)
                att_sb = ppool.tile([2, 16], F32)
                nc.vector.tensor_scalar_mul(att_sb, att_e, rec)

                # broadcast att rows across 128 partitions via PE
                for b in range(2):
                    ps_bc = psA.tile([128, 16], F32, tag="att_ps", name="ps_bc")
                    nc.tensor.matmul(
                        ps_bc, sel2[:, 128 * b : 128 * (b + 1)], att_sb,
                        start=True, stop=True,
                    )
                    abc = ppool.tile([128, 16], F32, name=f"att_bc{b}")
                    nc.vector.tensor_copy(abc, ps_bc)
                    att_bc.append(abc)

                # aggregated bias: bias_b[:, b] = sum_k att[b,k] bias[k, :]
                ps_at = psA.tile([16, 2], F32, tag="att_ps", name="ps_at")
                nc.tensor.transpose(ps_at, att_sb, id_f32[:2, :2])
                attT = ppool.tile([16, 2], F32)
                nc.vector.tensor_copy(attT, ps_at)
                ps_ab = psA.tile([128, 2], F32, tag="att_ps", name="ps_ab")
                nc.tensor.matmul(ps_ab, bias_sb2, attT, start=True, stop=True)
                nc.vector.tensor_copy(bias_b, ps_ab)

            # ---------------- weight mix (DVE, fused STT) ----------------
            # acc[b][ci][il, t, o] = sum_k att[b,k] * Wt[k, ci, t, il, o]
            acc = [[None, None], [None, None]]
            for ci in range(2):
                for b in range(2):
                    a = ppool.tile([128, 9, 128], BF16, name=f"acc{b}{ci}")
                    acc[b][ci] = a
                    for k in range(16):
                        wk = wsb[ci][:, k // 8, k % 8]        # [128, 9, 128]
                        if k == 0:
                            nc.vector.tensor_scalar_mul(a, wk, att_bc[b][:, 0:1])
                        else:
                            nc.vector.scalar_tensor_tensor(
                                a, wk, att_bc[b][:, k : k + 1], a,
                                op0=OP.mult, op1=OP.add,
                            )

            # ---------------- conv passes + epilogue ----------------
            astage = [[None] * 8, [None] * 8]
            with tc.tile_pool(name="psC", bufs=8, space="PSUM") as psC:
                # pass A: ci=0 taps, evacuate to SBUF staging
                for b in range(2):
                    pcv = {}
                    for t in range(9):
                        ty, tx = t // 3, t % 3
                        for pt in range(8):
                            r0 = 8 * pt
                            if t == 0:
                                pcv[pt] = psC.tile([128, 512], F32, tag="cv", name="pcv")
                            nc.tensor.matmul(
                                pcv[pt],
                                acc[b][0][:, t, :],
                                xpad[b][0][:, r0 + ty : r0 + ty + 8, tx : tx + 64],
                                start=(t == 0),
                                stop=(t == 8),
                            )
                    for pt in range(8):
                        stg = apool.tile([128, 512], F32, tag="astg", name="astg")
                        nc.scalar.activation(stg, pcv[pt], ACT.Identity)
                        astage[b][pt] = stg

                # pass B: ci=1 taps + epilogue
                for b in range(2):
                    pcv = {}
                    for t in range(9):
                        ty, tx = t // 3, t % 3
                        for pt in range(8):
                            r0 = 8 * pt
                            if t == 0:
                                pcv[pt] = psC.tile([128, 512], F32, tag="cv", name="pcv")
                            nc.tensor.matmul(
                                pcv[pt],
                                acc[b][1][:, t, :],
                                xpad[b][1][:, r0 + ty : r0 + ty + 8, tx : tx + 64],
                                start=(t == 0),
                                stop=(t == 8),
                            )
                    for pt in range(8):
                        r0 = 8 * pt
                        osb = opool.tile([128, 8, 64], F32, tag="osb", name="osb")
                        nc.scalar.activation(
                            osb, pcv[pt].rearrange("p (r c) -> p r c", r=8),
                            ACT.Identity, bias=bias_b[:, b : b + 1],
                        )
                        nc.vector.tensor_add(
                            osb,
                            osb,
                            astage[b][pt].rearrange("p (r c) -> p r c", r=8),
                        )
                        # residual: x bf16 lives in xpad[b][0] interior
                        nc.gpsimd.tensor_tensor(
                            osb,
                            osb,
                            xpad[b][0][:, 1 + r0 : 1 + r0 + 8, 1:65],
                            OP.add,
                        )
                        nc.scalar.dma_start(out=out2[b, :, r0 : r0 + 8, :], in_=osb)

    _split_multiwaits(nc)
    return nc


def _split_multiwaits(nc: bass.Bass):
    """This walrus build gives every TPB instruction exactly ONE sync-wait
    slot.  Tile emits multi-wait instructions; split the extras onto
    same-engine NoOp carriers inserted immediately before."""
    import bass_rust

    cnt = 0
    for fn in nc.m.functions:
        for blk in fn.blocks:
            out = []
            for ins in blk.instructions:
                si = getattr(ins, "sync_info", None)
                if si is not None and len(si.on_wait) > 1:
                    waits = list(si.on_wait)
                    for w in waits[:-1]:
                        cnt += 1
                        out.append(
                            bass_rust.InstNoOp(
                                name=f"waitcarrier-{cnt}",
                                engine=ins.engine,
                                ins=[],
                                outs=[],
                                sync_info=mybir.SyncInfo(
                                    on_wait=[w], on_update=[]
                                ),
                            )
                        )
                    ins.sync_info = mybir.SyncInfo(
                        on_wait=[waits[-1]], on_update=list(si.on_update)
                    )
                out.append(ins)
            blk.instructions = out


_PROGRAM = None


def _get_program():
    global _PROGRAM
    if _PROGRAM is None:
        _PROGRAM = build_program()
    return _PROGRAM


def _prepare_in_maps(x, scene_knowledge, weight, bias, att_w1, att_w2):
    x = np.ascontiguousarray(x, dtype=np.float32)
    scene_knowledge = np.ascontiguousarray(scene_knowledge, dtype=np.float32)
    weight = np.ascontiguousarray(weight, dtype=np.float32)
    bias = np.ascontiguousarray(bias, dtype=np.float32)
    att_w1 = np.ascontiguousarray(att_w1, dtype=np.float32)
    att_w2 = np.ascontiguousarray(att_w2, dtype=np.float32)

    # x padded to bf16 [bs, 2chunk, 128, 66, 66]
    xpadded = np.zeros((8, 2, 128, 66, 66), dtype=BF)
    xpadded[:, :, :, 1:65, 1:65] = x.reshape(8, 2, 128, 64, 64).astype(BF)

    # skp[g][r, b, dr, cc, dc] = scene[2g+b, 0, 2r+dr, 2cc+dc]
    skp_all = scene_knowledge.reshape(8, 28, 2, 28, 2)

    # w1rr[r, cc, j] = 0.25 * att_w1[j, r*28+cc]
    w1rr = np.ascontiguousarray(
        (0.25 * att_w1.T).reshape(28, 28, 196), dtype=BF
    )
    # w2r[p, c2, e] = att_w2[e, c2*98 + p]
    w2r = np.ascontiguousarray(att_w2.T.reshape(2, 98, 16).transpose(1, 0, 2))

    sel = np.zeros((2, 256), np.float32)
    sel[0, :128] = 1.0
    sel[1, 128:] = 1.0

    # wtb per h-half (2 distinct variants):
    # wtb[ci, kh, il, k2, t, o] = weight[kh*8+k2, 128h+o, 128*perm[ci]+il, t]
    wtb_h = []
    for h in range(2):
        perm = [h, 1 - h]
        w6 = weight.reshape(16, 2, 128, 2, 128, 9)[:, h]   # k, o, ih, il, t
        w6 = w6[:, :, perm]                                # k, o, ci, il, t
        w6 = w6.reshape(2, 8, 128, 2, 128, 9)              # kh, k2, o, ci, il, t
        wtb = np.ascontiguousarray(
            w6.transpose(3, 0, 4, 1, 5, 2), dtype=BF
        )                                                  # ci, kh, il, k2, t, o
        wtb_h.append(wtb)

    biash_h = [
        np.ascontiguousarray(bias[:, 128 * h : 128 * (h + 1)]) for h in range(2)
    ]

    in_maps = []
    for c in range(NCORES):
        g, h = c // 2, c % 2
        perm = [h, 1 - h]
        xc = np.ascontiguousarray(xpadded[2 * g : 2 * g + 2, perm])
        # skp[r, b, dr, cc, dc] = scene[2g+b, 0, 2r+dr, 2cc+dc]
        skc = np.ascontiguousarray(
            skp_all[2 * g : 2 * g + 2].transpose(1, 0, 2, 3, 4)
        )
        in_maps.append(
            {
                "xp": xc,
                "skp": skc,
                "w1rr": w1rr,
                "w2r": w2r,
                "wtb": wtb_h[h],
                "biash": biash_h[h],
                "selc": sel,
            }
        )
    return in_maps


def _assemble(results):
    out = np.empty((8, 256, 64, 64), np.float32)
    for c in range(NCORES):
        g, h = c // 2, c % 2
        out[2 * g : 2 * g + 2, 128 * h : 128 * (h + 1)] = results[c]["out2"]
    return out


def run(inputs: dict, trace: bool = False, tmpdir: str | None = None):
    from concourse.bass_utils import run_bass_kernel_spmd

    nc = _get_program()
    in_maps = _prepare_in_maps(**inputs)
    res = run_bass_kernel_spmd(
        nc, in_maps, core_ids=list(range(NCORES)), trace=trace, tmpdir=tmpdir
    )
    return _assemble(res.results), res


def kernel(**inputs) -> np.ndarray:
    out, _ = run(inputs, trace=False)
    return out


# revision 5
# speedup vs baseline: 1.6041x; 1.1972x over previous
"""Trainium2 Bass kernel for the CondConv-style dense CNN.

Model (per sample b):
  att[b]  = softmax(MLP(avgpool(scene_knowledge[b])) / 30)        # [16]
  agg_w   = sum_k att[b,k] * weight[k]                            # [256,256,3,3]
  out[b]  = conv3x3_same(x[b], agg_w) + att[b] @ bias + x[b]

Sharding: 8 cores = 4 sample-pairs (g) x 2 output-channel halves (h).
Each core processes 2 samples and 128 output channels.

v3 design:
  - Host pre-casts W to bf16 in conv-ready layout and pre-pads x to the
    bf16 [66,66] padded layout -> all loads are plain HWDGE DMAs.
  - DMA rings: weight slabs alone on the sync ring; everything else
    (attention smalls first, then x) on the scalar ring.
  - Attention: 112-partition pooling layout -> 7 hidden matmuls; softmax
    without max-subtraction (logits/30 are tiny); exp reads PSUM.
  - Weight mix: ACT produces att_k-scaled slabs (activation Copy with
    scale), DVE runs pure tensor_tensor add chains (2x mode).
  - Conv: pass A (ci=0) over all 16 (b,pt) tiles, evacuated by ACT with
    the bias folded in; GpSimd folds the residual into the staging
    buffer; pass B (ci=1) accumulates and DVE merges PSUM + staging.
  - PE warm-up matmuls keep the clock at 2.4 GHz before pass A.
"""

import sys
import numpy as np

sys.path.insert(0, "/opt/trn_rl_repo")

import ml_dtypes
import concourse.bass as bass
import concourse.mybir as mybir
from concourse.tile import TileContext
from concourse.masks import make_identity

F32 = mybir.dt.float32
BF16 = mybir.dt.bfloat16
AX = mybir.AxisListType
OP = mybir.AluOpType
ACT = mybir.ActivationFunctionType

TEMPERATURE = 30.0
NCORES = 8
BF = ml_dtypes.bfloat16


def build_program() -> bass.Bass:
    nc = bass.Bass()

    # [b, ci, il, 66, 66] zero-padded bf16 x (ci=0 == this core's out half)
    xp = nc.declare_dram_parameter("xp", [2, 2, 128, 66, 66], BF16, isOutput=False)
    # scene knowledge packed for on-device 2x2 pooling on 112 partitions:
    # skp[p=(r,c4), b, dr, c7, dc] = scene[b, 0, 2r+dr, 2*(c4*7+c7)+dc]
    skp = nc.declare_dram_parameter("skp", [112, 2, 2, 7, 2], F32, isOutput=False)
    # w1rr[p=(r,c4), c7, j] = 0.25 * att_w1[j, r*28 + c4*7 + c7]
    w1rr = nc.declare_dram_parameter("w1rr", [112, 7, 196], BF16, isOutput=False)
    # w2r[p, c2, e] = att_w2[e, c2*98 + p]
    w2r = nc.declare_dram_parameter("w2r", [98, 2, 16], F32, isOutput=False)
    # wtb[ci, kh, il, k2, t, o] = weight[kh*8+k2, 128h+o, chan(ci,il), t]
    wtb = nc.declare_dram_parameter("wtb", [2, 2, 128, 8, 9, 128], BF16, isOutput=False)
    biash = nc.declare_dram_parameter("biash", [16, 128], F32, isOutput=False)
    selc = nc.declare_dram_parameter("selc", [2, 256], F32, isOutput=False)
    out2 = nc.declare_dram_parameter("out2", [2, 128, 64, 64], F32, isOutput=True)

    with TileContext(nc) as tc:
        with (
            tc.tile_pool(name="const", bufs=1) as cpool,
            tc.tile_pool(name="persist", bufs=1) as ppool,
            tc.tile_pool(name="wmix", bufs=4) as tpool,
            tc.tile_pool(name="astage", bufs=16) as apool,
            tc.tile_pool(name="outstage", bufs=4) as opool,
        ):
            # ---------------- DMAs (two HWDGE rings) ----------------
            # sync ring: the 4 weight slabs, nothing else ahead of them.
            wsb = []
            for ci in range(2):
                w = ppool.tile([128, 2, 8, 9, 128], BF16, name=f"wsb{ci}")
                for kh in range(2):
                    nc.sync.dma_start(out=w[:, kh], in_=wtb[ci, kh])
                wsb.append(w)

            # scalar ring: attention smalls first, then the padded x slabs.
            sk_sb = cpool.tile([112, 2, 2, 7, 2], F32)
            nc.scalar.dma_start(out=sk_sb, in_=skp[:])
            w1_sb = cpool.tile([112, 7, 196], BF16)
            nc.scalar.dma_start(out=w1_sb, in_=w1rr[:])
            w2_sb = cpool.tile([98, 2, 16], F32)
            nc.scalar.dma_start(out=w2_sb, in_=w2r[:])
            sel = cpool.tile([2, 256], F32)
            nc.scalar.dma_start(out=sel, in_=selc[:])
            bias_sb = cpool.tile([16, 128], F32)
            nc.scalar.dma_start(out=bias_sb, in_=biash[:])
            xpad = [[None, None], [None, None]]
            for ci in range(2):
                for b in range(2):
                    t = ppool.tile([128, 66, 66], BF16, name=f"xp{b}{ci}")
                    nc.scalar.dma_start(out=t, in_=xp[b, ci])
                    xpad[b][ci] = t

            # DVE twins so PE matmuls get single-engine deps.
            sel2 = cpool.tile([2, 256], F32)
            nc.vector.tensor_copy(sel2, sel)
            bias_sb2 = cpool.tile([16, 128], F32)
            nc.vector.tensor_copy(bias_sb2, bias_sb)

            id_f32 = cpool.tile([16, 16], F32)
            make_identity(nc, id_f32)

            att_bc = []  # [128, 16] f32 per sample
            bias_b = ppool.tile([128, 2], F32)

            # ---------------- attention ----------------
            with tc.tile_pool(name="psA", bufs=2, space="PSUM") as psA:
                # 2x2 avg pool on DVE (0.25 folded into w1rr)
                pool_a = ppool.tile([112, 2, 7, 2], F32)
                nc.vector.tensor_add(pool_a, sk_sb[:, :, 0], sk_sb[:, :, 1])
                pooled = ppool.tile([112, 2, 7], BF16)
                nc.vector.tensor_add(pooled, pool_a[:, :, :, 0], pool_a[:, :, :, 1])

                # hidden = relu(pooled @ w1.T): 7 accumulating matmuls
                ps_h = psA.tile([2, 196], F32, tag="att_ps")
                for c7 in range(7):
                    nc.tensor.matmul(
                        ps_h,
                        pooled[:, :, c7],        # [112, 2]
                        w1_sb[:, c7, :],         # [112, 196]
                        start=(c7 == 0),
                        stop=(c7 == 6),
                    )
                hdn = ppool.tile([2, 196], F32)
                nc.vector.tensor_relu(hdn, ps_h)

                # transpose hdn chunks: [2, 98] -> [98, 2]
                hdnT = ppool.tile([98, 2, 2], F32)
                for c2 in range(2):
                    ps_t = psA.tile([98, 2], F32, tag="att_ps", name="ps_t")
                    nc.tensor.transpose(
                        ps_t, hdn[:, 98 * c2 : 98 * (c2 + 1)], id_f32[:2, :2]
                    )
                    nc.vector.tensor_copy(hdnT[:, c2, :], ps_t)

                # logits = hdn @ w2.T -> [2, 16]
                ps_l = psA.tile([2, 16], F32, tag="att_ps", name="ps_l")
                for c2 in range(2):
                    nc.tensor.matmul(
                        ps_l,
                        hdnT[:, c2, :],
                        w2_sb[:, c2, :],
                        start=(c2 == 0),
                        stop=(c2 == 1),
                    )

                # softmax(logits / T): logits/T is tiny, no max-sub needed
                att_e = ppool.tile([2, 16], F32)
                nc.scalar.activation(att_e, ps_l, ACT.Exp, scale=1.0 / TEMPERATURE)
                sm = ppool.tile([2, 1], F32)
                nc.vector.tensor_reduce(sm, att_e, axis=AX.X, op=OP.add)
                rec = ppool.tile([2, 1], F32)
                nc.vector.reciprocal(rec, sm)
                att_sb = ppool.tile([2, 16], F32)
                nc.vector.tensor_scalar_mul(att_sb, att_e, rec)

                # broadcast att rows across 128 partitions via PE
                for b in range(2):
                    ps_bc = psA.tile([128, 16], F32, tag="att_ps", name="ps_bc")
                    nc.tensor.matmul(
                        ps_bc, sel2[:, 128 * b : 128 * (b + 1)], att_sb,
                        start=True, stop=True,
                    )
                    abc = ppool.tile([128, 16], F32, name=f"att_bc{b}")
                    nc.vector.tensor_copy(abc, ps_bc)
                    att_bc.append(abc)

                # aggregated bias: bias_b[:, b] = sum_k att[b,k] bias[k, :]
                ps_at = psA.tile([16, 2], F32, tag="att_ps", name="ps_at")
                nc.tensor.transpose(ps_at, att_sb, id_f32[:2, :2])
                attT = ppool.tile([16, 2], F32)
                nc.vector.tensor_copy(attT, ps_at)
                ps_ab = psA.tile([128, 2], F32, tag="att_ps", name="ps_ab")
                nc.tensor.matmul(ps_ab, bias_sb2, attT, start=True, stop=True)
                nc.vector.tensor_copy(bias_b, ps_ab)

            # ---------------- weight mix: ACT muls + DVE add chains -------
            # acc[b][ci][il, t, o] = sum_k att[b,k] * Wt[k, ci, t, il, o]
            acc = [[None, None], [None, None]]

            def mix_chain(ci, b):
                a = ppool.tile([128, 9, 128], BF16, name=f"acc{b}{ci}")
                acc[b][ci] = a
                for k in range(16):
                    wk = wsb[ci][:, k // 8, k % 8]            # [128, 9, 128]
                    if k == 0:
                        nc.scalar.mul(a, wk, att_bc[b][:, 0:1])
                    else:
                        tmp = tpool.tile(
                            [128, 9, 128], BF16, tag="wtmp", name="wtmp"
                        )
                        nc.scalar.mul(tmp, wk, att_bc[b][:, k : k + 1])
                        nc.vector.tensor_add(a, a, tmp)

            mix_chain(0, 0)
            mix_chain(0, 1)

            # ---------------- conv passes + epilogue ----------------
            # Emission order is engine-stream order.  DVE stream must be:
            # [ci0 adds][ci1b0 adds][b0 evacs][ci1b1 adds][b1 evacs][merges]
            # so evacuations never starve the mix chains.
            astage = [[None] * 8, [None] * 8]
            pcvA = [{}, {}]
            with tc.tile_pool(name="psW", bufs=1, space="PSUM") as psW:
                # PE warm-up: junk matmuls on the first weight slab keep the
                # clock-gate at 8/8 before pass A starts.
                junk = psW.tile([128, 512], F32, tag="warm")
                for i in range(20):
                    nc.tensor.matmul(
                        junk,
                        wsb[0][:, 0, 0, 0, :],
                        wsb[0][:, 0, i % 8, 0:4, :],
                        start=True,
                        stop=True,
                    )

            with tc.tile_pool(name="psC", bufs=8, space="PSUM") as psC:

                def pass_mm(b, ci, pcv):
                    for t in range(9):
                        ty, tx = t // 3, t % 3
                        for pt in range(8):
                            r0 = 8 * pt
                            if t == 0:
                                pcv[pt] = psC.tile(
                                    [128, 512], F32, tag="cv", name="pcv"
                                )
                            nc.tensor.matmul(
                                pcv[pt],
                                acc[b][ci][:, t, :],
                                xpad[b][ci][:, r0 + ty : r0 + ty + 8, tx : tx + 64],
                                start=(t == 0),
                                stop=(t == 8),
                            )

                def evac(b):
                    # DVE: stg = passA + bias; GpSimd: stg += residual x
                    for pt in range(8):
                        stg = apool.tile([128, 8, 64], F32, tag="astg", name="astg")
                        nc.vector.tensor_scalar_add(
                            stg,
                            pcvA[b][pt].rearrange("p (r c) -> p r c", r=8),
                            bias_b[:, b : b + 1],
                        )
                        r0 = 8 * pt
                        nc.gpsimd.tensor_tensor(
                            stg, stg, xpad[b][0][:, 1 + r0 : 1 + r0 + 8, 1:65], OP.add
                        )
                        astage[b][pt] = stg

                pass_mm(0, 0, pcvA[0])       # PE: A-b0
                pass_mm(1, 0, pcvA[1])       # PE: A-b1
                mix_chain(1, 0)              # DVE: ci1b0 adds (+ACT muls)
                evac(0)                      # DVE/GpSimd: b0 evac+residual
                mix_chain(1, 1)              # DVE: ci1b1 adds
                evac(1)                      # DVE/GpSimd: b1 evac+residual

                # pass B: ci=1 taps; DVE merges PSUM + staging; DMA out
                for b in range(2):
                    pcv = {}
                    pass_mm(b, 1, pcv)
                    for pt in range(8):
                        r0 = 8 * pt
                        osb = opool.tile([128, 8, 64], F32, tag="osb", name="osb")
                        nc.vector.tensor_add(
                            osb,
                            pcv[pt].rearrange("p (r c) -> p r c", r=8),
                            astage[b][pt],
                        )
                        nc.scalar.dma_start(out=out2[b, :, r0 : r0 + 8, :], in_=osb)

    _split_multiwaits(nc)
    return nc


def _split_multiwaits(nc: bass.Bass):
    """This walrus build gives every TPB instruction exactly ONE sync-wait
    slot.  Tile emits multi-wait instructions; split the extras onto
    same-engine NoOp carriers inserted immediately before."""
    import bass_rust

    cnt = 0
    for fn in nc.m.functions:
        for blk in fn.blocks:
            out = []
            for ins in blk.instructions:
                si = getattr(ins, "sync_info", None)
                if si is not None and len(si.on_wait) > 1:
                    waits = list(si.on_wait)
                    for w in waits[:-1]:
                        cnt += 1
                        out.append(
                            bass_rust.InstNoOp(
                                name=f"waitcarrier-{cnt}",
                                engine=ins.engine,
                                ins=[],
                                outs=[],
                                sync_info=mybir.SyncInfo(
                                    on_wait=[w], on_update=[]
                                ),
                            )
                        )
                    ins.sync_info = mybir.SyncInfo(
                        on_wait=[waits[-1]], on_update=list(si.on_update)
                    )
                out.append(ins)
            blk.instructions = out


_PROGRAM = None


def _get_program():
    global _PROGRAM
    if _PROGRAM is None:
        _PROGRAM = build_program()
    return _PROGRAM


def _prepare_in_maps(x, scene_knowledge, weight, bias, att_w1, att_w2):
    x = np.ascontiguousarray(x, dtype=np.float32)
    scene_knowledge = np.ascontiguousarray(scene_knowledge, dtype=np.float32)
    weight = np.ascontiguousarray(weight, dtype=np.float32)
    bias = np.ascontiguousarray(bias, dtype=np.float32)
    att_w1 = np.ascontiguousarray(att_w1, dtype=np.float32)
    att_w2 = np.ascontiguousarray(att_w2, dtype=np.float32)

    # x padded to bf16 [bs, 2chunk, 128, 66, 66]
    xpadded = np.zeros((8, 2, 128, 66, 66), dtype=BF)
    xpadded[:, :, :, 1:65, 1:65] = x.reshape(8, 2, 128, 64, 64).astype(BF)

    # skp[p=(r,c4), b, dr, c7, dc] = scene[b, 0, 2r+dr, 2*(c4*7+c7)+dc]
    # scene [bs, 1, 56, 56] -> [bs, 28, 2, 4, 7, 2] = (b, r, dr, c4, c7, dc)
    sk6 = scene_knowledge.reshape(8, 28, 2, 4, 7, 2)

    # w1rr[p=(r,c4), c7, j] = 0.25 * att_w1[j, r*28 + c4*7 + c7]
    w1rr = np.ascontiguousarray(
        (0.25 * att_w1.T).reshape(112, 7, 196), dtype=BF
    )
    # w2r[p, c2, e] = att_w2[e, c2*98 + p]
    w2r = np.ascontiguousarray(att_w2.T.reshape(2, 98, 16).transpose(1, 0, 2))

    sel = np.zeros((2, 256), np.float32)
    sel[0, :128] = 1.0
    sel[1, 128:] = 1.0

    # wtb per h-half (2 distinct variants):
    wtb_h = []
    for h in range(2):
        perm = [h, 1 - h]
        w6 = weight.reshape(16, 2, 128, 2, 128, 9)[:, h]   # k, o, ih, il, t
        w6 = w6[:, :, perm]                                # k, o, ci, il, t
        w6 = w6.reshape(2, 8, 128, 2, 128, 9)              # kh, k2, o, ci, il, t
        wtb = np.ascontiguousarray(
            w6.transpose(3, 0, 4, 1, 5, 2), dtype=BF
        )                                                  # ci, kh, il, k2, t, o
        wtb_h.append(wtb)

    biash_h = [
        np.ascontiguousarray(bias[:, 128 * h : 128 * (h + 1)]) for h in range(2)
    ]

    in_maps = []
    for c in range(NCORES):
        g, h = c // 2, c % 2
        perm = [h, 1 - h]
        xc = np.ascontiguousarray(xpadded[2 * g : 2 * g + 2, perm])
        # [r, c4, b, dr, c7, dc] -> merge (r, c4) into 112 partitions
        skc = np.ascontiguousarray(
            sk6[2 * g : 2 * g + 2].transpose(1, 3, 0, 2, 4, 5).reshape(
                112, 2, 2, 7, 2
            )
        )
        in_maps.append(
            {
                "xp": xc,
                "skp": skc,
                "w1rr": w1rr,
                "w2r": w2r,
                "wtb": wtb_h[h],
                "biash": biash_h[h],
                "selc": sel,
            }
        )
    return in_maps


def _assemble(results):
    out = np.empty((8, 256, 64, 64), np.float32)
    for c in range(NCORES):
        g, h = c // 2, c % 2
        out[2 * g : 2 * g + 2, 128 * h : 128 * (h + 1)] = results[c]["out2"]
    return out


def run(inputs: dict, trace: bool = False, tmpdir: str | None = None):
    from concourse.bass_utils import run_bass_kernel_spmd

    nc = _get_program()
    in_maps = _prepare_in_maps(**inputs)
    res = run_bass_kernel_spmd(
        nc, in_maps, core_ids=list(range(NCORES)), trace=trace, tmpdir=tmpdir
    )
    return _assemble(res.results), res


def kernel(**inputs) -> np.ndarray:
    out, _ = run(inputs, trace=False)
    return out
